# revision 24
# baseline (speedup 1.0000x reference)
"""Trainium2 Bass kernel for nn_MultiHeadAttention_41944650612760.

Wasserstein-distance multi-head attention with cumulative position decay.
Sharding: data-parallel over batch B=8 across 8 NeuronCores (one batch/core).

Design notes:
  - fp16 matmuls everywhere (P1 projections, scores, PV, P4).
  - one ACT table set per phase: P1 issues all Sqrt work first, then only
    filler funcs (Identity/Square/Copy), so the switch to P3's
    natural_log_exp_and_others set happens exactly once; the position-decay
    sqrt runs in the ln-domain (sqrt(x) = exp(0.5 ln x)).
  - causal btri mask added on PE via identity-matmul; b1 column term added
    via K=1 ones-matmul; both folded into the scores PSUM accumulation.
  - suffix mass S computed directly by a REVERSED DVE scan (no sm1 - C
    cancellation), fp16 out; F'' = S * (g^2/sm1) * |t-s| in one fused stt.
  - decay chain (Ln, Exp, Exp) batched across 4-head halves.
  - second-softmax numerator kept fp32 until normalize, then fp16 for the
    transpose + PV; PV accumulates per row-block into packed PSUM slots
    (mean and cov groups sequential: start=True clears the whole bank).
  - P4 output projection runs per row-block, pipelined with P3 via per-tb
    DRAM bounce tiles.

zero_pad is applied on the host (row 0 of each output = bias).
"""

import os
import numpy as np
from contextlib import ExitStack

B, T, D, H = 8, 1024, 512, 8
DK = D // H          # 64
NT = T // 128        # 8 row/col blocks
NEG = -30000.0       # fp16-safe mask constant (x0.125 => -3750)
F16 = np.float16

# packed row-block layout for the |t-s| table: block tb holds s in
# [0, (tb+1)*128) at DOFF[tb]
DOFF = [0] * NT
for _t in range(1, NT):
    DOFF[_t] = DOFF[_t - 1] + _t * 128
DD_COLS = DOFF[-1] + NT * 128  # 4608


def _build(gamma2):
    """Trace the Bass program. gamma2[h] = gamma_h**2 (trace-time floats)."""
    import concourse.bass as bass
    import concourse.bacc as bacc
    import concourse.mybir as mybir
    import concourse.tile as tile

    # Steer the ACT table chooser: drop `exp` from exp_and_others and `ln`
    # from natural_log so both resolve to natural_log_exp_and_others (one
    # resident set for all of P3/P4 instead of per-Ln flip-flops). Set IDs
    # stay positional, so the emitted act_func_set_id values remain valid.
    import concourse.hw_specs as hw_specs
    if not getattr(bacc, "_act_tbl_patched", False):
        _orig_gat = hw_specs.get_activation_tables

        def _gat(arch):
            tables = _orig_gat(arch)
            exp_t = mybir.ActivationFunctionType.from_pwp("exp")
            ln_t = mybir.ActivationFunctionType.from_pwp("ln")
            if "natural_log_exp_and_others" in tables:
                tables.get("exp_and_others", set()).discard(exp_t)
                tables.get("natural_log", set()).discard(ln_t)
            return tables

        bacc.get_activation_tables = _gat
        bacc._act_tbl_patched = True

    dt = mybir.dt
    AF = mybir.ActivationFunctionType
    OP = mybir.AluOpType
    ts = bass.ts

    nc = bacc.Bacc()

    # ---- per-core DRAM I/O ----
    xT = nc.declare_dram_parameter("xT", [6, D, T], dt.float16, isOutput=False)
    wT = nc.declare_dram_parameter("wT", [4, D, D], dt.float16, isOutput=False)
    woT = nc.declare_dram_parameter("woT", [2, D, D], dt.float16, isOutput=False)
    wc = nc.declare_dram_parameter("wc", [D, H], dt.float16, isOutput=False)
    bqk = nc.declare_dram_parameter("bqk", [128, 12], dt.float32, isOutput=False)
    bvp = nc.declare_dram_parameter("bvp", [128, 8], dt.float32, isOutput=False)
    bvo16 = nc.declare_dram_parameter("bvo16", [2, D], dt.float16, isOutput=False)
    cb = nc.declare_dram_parameter("cb", [8, 2], dt.float32, isOutput=False)
    cbr = nc.declare_dram_parameter("cbr", [1, 8], dt.float32, isOutput=False)
    btri = nc.declare_dram_parameter("btri", [128, 128], dt.float16, isOutput=False)
    ident = nc.declare_dram_parameter("ident", [128, 128], dt.float16, isOutput=False)
    dtab = nc.declare_dram_parameter("dtab", [128, DD_COLS], dt.float16, isOutput=False)
    selh_in = nc.declare_dram_parameter("selh_in", [8, H * 128], dt.float16, isOutput=False)
    out_m = nc.declare_dram_parameter("out_m", [T, D], dt.float32, isOutput=True)
    out_c = nc.declare_dram_parameter("out_c", [T, D], dt.float32, isOutput=True)

    def rev(a):
        """Reverse an AP [128, W] along its (last) free dim."""
        st, cnt = a.ap[-1]
        return bass.AP(tensor=a.tensor, offset=a.offset + (cnt - 1) * st,
                       ap=list(a.ap[:-1]) + [[-st, cnt]])

    with tile.TileContext(nc) as tc, ExitStack() as ctx:
        pc = ctx.enter_context(tc.tile_pool(name="pc", bufs=1))
        pdr = ctx.enter_context(tc.tile_pool(name="pdr", bufs=1, space="DRAM"))

        # ---- persistent SBUF tensors ----
        U2 = pc.tile([128, H, T], dt.float16)       # [qm_h ; sqq_h] per head (parity split)
        W2 = pc.tile([128, H, T], dt.float16)       # [2km_h ; 2sqk_h]
        vm16 = pc.tile([128, NT, D], dt.float16)    # vm normal layout fp16
        vc16 = pc.tile([128, NT, D], dt.float16)
        btri_sb = pc.tile([128, 128], dt.float16)
        ident_sb = pc.tile([128, 128], dt.float16)
        ones16 = pc.tile([1, T], dt.float16)
        a1n8 = pc.tile([128, NT, 8], dt.float32)    # -0.125 * (a1[t]+sb) per (tb, h)
        cbn8 = pc.tile([128, 8], dt.float32)        # -0.125 * sb, bcast to 128 parts
        bqk_sb = pc.tile([128, 12], dt.float32)     # [bk|bkc_sw|2bk] x 4 chunks
        bvp_sb = pc.tile([128, 8], dt.float32)      # bv, bvc pair-sliced
        cb_sb = pc.tile([8, 2], dt.float32)
        wc_sb = pc.tile([128, 4, H], dt.float16)
        E2q = pc.tile([128, 4, 8], dt.float16)
        E2k = pc.tile([128, 4, 8], dt.float16)
        wo16 = pc.tile([128, 2, 4, D], dt.float16)  # P4 weights, both outputs
        bo16 = pc.tile([1, 2, D], dt.float16)
        b1all = pc.tile([8, T], dt.float16)         # -b1 rows, partitions 0-7
        selh = pc.tile([8, H, 128], dt.float16)     # one-hot row selectors (K=8 MM)

        b_dr = pdr.tile([H, T], dt.float16)         # -b1 rows via DRAM bounce
        cm_drs = [pdr.tile([2, D, 128], dt.float16, name=f"cm{tb}")
                  for tb in range(NT)]              # per-tb attention-out bounce

        nc.sync.dma_start(out=btri_sb, in_=btri[:, :])
        nc.sync.dma_start(out=ident_sb, in_=ident[:, :])
        nc.sync.dma_start(out=bqk_sb, in_=bqk[:, :])
        nc.sync.dma_start(out=bvp_sb, in_=bvp[:, :])
        nc.sync.dma_start(out=cb_sb, in_=cb[:, :])
        nc.sync.dma_start(out=wc_sb, in_=wc.rearrange("(k p) h -> p k h", p=128))
        nc.sync.dma_start(out=wo16, in_=woT.rearrange("i (k p) d -> p i k d", p=128))
        _bv = bvo16[:, :]
        nc.sync.dma_start(out=bo16, in_=bass.AP(tensor=_bv.tensor, offset=_bv.offset,
                                                ap=[[0, 1]] + list(_bv.ap)))
        nc.vector.memset(ones16, 1.0)
        _cbr = cbr[0:1, :]
        nc.sync.dma_start(out=cbn8, in_=bass.AP(tensor=_cbr.tensor, offset=_cbr.offset,
                                                ap=[[0, 128]] + list(_cbr.ap[1:])))
        nc.sync.dma_start(out=selh, in_=selh_in.rearrange("p (h c) -> p h c", h=H))
        nc.vector.memset(E2q, 0.0)
        nc.vector.memset(E2k, 0.0)
        for c in range(4):
            nc.vector.memset(E2q[0:64, c, 2 * c:2 * c + 1], 1.0)
            nc.vector.memset(E2q[64:128, c, 2 * c + 1:2 * c + 2], 1.0)
            nc.vector.memset(E2k[0:64, c, 2 * c:2 * c + 1], 0.25)
            nc.vector.memset(E2k[64:128, c, 2 * c + 1:2 * c + 2], 0.25)

        # =================== P1: projections + P2: a1/b1 ===================
        # Order: cov runs (2, 4) FIRST so all ACT Sqrt work precedes the
        # P3 exp/ln table set (exactly one ACT_TABLE_LOAD switch).
        with tc.tile_pool(name="p1x", bufs=6) as px, \
             tc.tile_pool(name="p1w", bufs=1) as pw, \
             tc.tile_pool(name="p1z", bufs=3) as pz, \
             tc.tile_pool(name="p1ps", bufs=2, space="PSUM") as pps, \
             tc.tile_pool(name="p1pa", bufs=1, space="PSUM") as ppa:

            b_ps = ppa.tile([8, T], dt.float32, tag="b_ps")   # b1 = m2sq + kcs
            a1t_ps = ppa.tile([128, NT, 8], dt.float32, tag="a1t")  # a1 in [t, (tb,h)]

            def load_x(i):
                xs = []
                for k in range(4):
                    xt = px.tile([128, T], dt.float16, tag="xt")
                    nc.sync.dma_start(out=xt, in_=xT[i, ts(k, 128), :])
                    xs.append(xt)
                return xs

            def load_w(i, tagslot):
                wt = pw.tile([128, 4, D], dt.float16, tag=f"wt{tagslot}")
                nc.sync.dma_start(out=wt, in_=wT[i].rearrange("(k p) d -> p k d", p=128))
                return wt

            # ---- run 2: sqq -> U2 (clip+sqrt) + qcs into a1t (group start) ----
            xqc = load_x(1)
            wkc = load_w(1, 1)
            for c in range(4):
                for n in range(2):
                    ps = pps.tile([128, 512], dt.float32, tag="ps")
                    for k in range(4):
                        nc.tensor.matmul(ps, wkc[:, k, ts(c, 128)],
                                         xqc[k][:, ts(n, 512)],
                                         start=(k == 0), stop=(k == 3))
                    nc.vector.tensor_scalar(out=ps, in0=ps, scalar1=bqk_sb[:, 4 + c:5 + c],
                                            scalar2=1e-24, op0=OP.add, op1=OP.max)
                    # swapped col order: psum[0:64] = head 2c+1 (odd -> low parts)
                    nc.scalar.activation(out=U2[0:64, 2 * c + 1, ts(n, 512)], in_=ps[0:64],
                                         func=AF.Sqrt)
                    nc.scalar.activation(out=U2[64:128, 2 * c, ts(n, 512)], in_=ps[64:128],
                                         func=AF.Sqrt)
            # single accumulation group for the whole a1t bank: start=True
            # clears has_written bank-wide, so only the very first MM starts
            for tb in range(NT):
                for k in range(4):
                    nc.tensor.matmul(a1t_ps[:, tb, :], xqc[k][:, ts(tb, 128)],
                                     wc_sb[:, k, :],
                                     start=(tb == 0 and k == 0), stop=False)

            # ---- run 4: 2sqk -> W2 + kcs into b_ps (group start) ----
            xkc = load_x(3)
            for n in range(2):
                for k in range(4):
                    nc.tensor.matmul(b_ps[:, ts(n, 512)], wc_sb[:, k, :],
                                     xkc[k][:, ts(n, 512)],
                                     start=(k == 0), stop=False)
            for c in range(4):
                for n in range(2):
                    ps = pps.tile([128, 512], dt.float32, tag="ps")
                    for k in range(4):
                        nc.tensor.matmul(ps, wkc[:, k, ts(c, 128)],
                                         xkc[k][:, ts(n, 512)],
                                         start=(k == 0), stop=(k == 3))
                    nc.vector.tensor_scalar(out=ps, in0=ps, scalar1=bqk_sb[:, 4 + c:5 + c],
                                            scalar2=1e-24, op0=OP.add, op1=OP.max)
                    nc.scalar.activation(out=W2[0:64, 2 * c + 1, ts(n, 512)], in_=ps[0:64],
                                         func=AF.Sqrt, scale=4.0)
                    nc.scalar.activation(out=W2[64:128, 2 * c, ts(n, 512)], in_=ps[64:128],
                                         func=AF.Sqrt, scale=4.0)

            # ---- run 1: qm -> U2 (bias bk; ACT/DVE split) ----
            xq = load_x(0)
            wk = load_w(0, 0)
            for c in range(4):
                for n in range(2):
                    ps = pps.tile([128, 512], dt.float32, tag="ps")
                    for k in range(4):
                        nc.tensor.matmul(ps, wk[:, k, ts(c, 128)],
                                         xq[k][:, ts(n, 512)],
                                         start=(k == 0), stop=(k == 3))
                    if n == 0:
                        nc.scalar.activation(out=U2[0:64, 2 * c, ts(n, 512)], in_=ps[0:64],
                                             func=AF.Identity, bias=bqk_sb[0:64, c:c + 1])
                        nc.scalar.activation(out=U2[64:128, 2 * c + 1, ts(n, 512)],
                                             in_=ps[64:128],
                                             func=AF.Identity, bias=bqk_sb[64:128, c:c + 1])
                    else:
                        nc.vector.tensor_scalar(out=U2[0:64, 2 * c, ts(n, 512)],
                                                in0=ps[0:64], scalar1=bqk_sb[0:64, c:c + 1],
                                                scalar2=None, op0=OP.add)
                        nc.vector.tensor_scalar(out=U2[64:128, 2 * c + 1, ts(n, 512)],
                                                in0=ps[64:128],
                                                scalar1=bqk_sb[64:128, c:c + 1],
                                                scalar2=None, op0=OP.add)

            # ---- run 3: 2km -> W2 (bias 2bk, scale 2; ACT/DVE split) ----
            xk = load_x(2)
            for c in range(4):
                for n in range(2):
                    ps = pps.tile([128, 512], dt.float32, tag="ps")
                    for k in range(4):
                        nc.tensor.matmul(ps, wk[:, k, ts(c, 128)],
                                         xk[k][:, ts(n, 512)],
                                         start=(k == 0), stop=(k == 3))
                    if n == 0:
                        nc.scalar.activation(out=W2[0:64, 2 * c, ts(n, 512)], in_=ps[0:64],
                                             func=AF.Identity, scale=2.0,
                                             bias=bqk_sb[0:64, 8 + c:9 + c])
                        nc.scalar.activation(out=W2[64:128, 2 * c + 1, ts(n, 512)],
                                             in_=ps[64:128], func=AF.Identity, scale=2.0,
                                             bias=bqk_sb[64:128, 8 + c:9 + c])
                    else:
                        nc.vector.tensor_scalar(out=W2[0:64, 2 * c, ts(n, 512)],
                                                in0=ps[0:64], scalar1=2.0,
                                                scalar2=bqk_sb[0:64, 8 + c:9 + c],
                                                op0=OP.mult, op1=OP.add)
                        nc.vector.tensor_scalar(out=W2[64:128, 2 * c + 1, ts(n, 512)],
                                                in0=ps[64:128], scalar1=2.0,
                                                scalar2=bqk_sb[64:128, 8 + c:9 + c],
                                                op0=OP.mult, op1=OP.add)

            # ---- q-side squares (m1sq) -> finish a1t group; evict a1n8 ----
            zqs = []
            for c in range(4):
                zq = pz.tile([128, T], dt.float16, tag=f"zq{c}", name="zq", bufs=1)
                nc.scalar.activation(out=zq[0:64, :], in_=U2[0:64, 2 * c, :], func=AF.Square)
                nc.scalar.activation(out=zq[64:128, :], in_=U2[64:128, 2 * c + 1, :],
                                     func=AF.Square)
                zqs.append(zq)
            for tb in range(NT):
                for c in range(4):
                    nc.tensor.matmul(a1t_ps[:, tb, :], zqs[c][:, ts(tb, 128)],
                                     E2q[:, c, :],
                                     start=False, stop=(tb == NT - 1 and c == 3))
            nc.scalar.activation(out=a1n8, in_=a1t_ps, func=AF.Copy, scale=-0.125)
            nc.vector.tensor_tensor(out=a1n8, in0=a1n8,
                                    in1=bass.AP(tensor=cbn8.tensor, offset=cbn8.offset,
                                                ap=[cbn8.ap[0], [0, NT]] + list(cbn8.ap[1:])),
                                    op=OP.add)

            # ---- k-side squares (of 2km; E2k carries the 1/4) -> b_ps; b1 rows ----
            for c in range(4):
                zk = pz.tile([128, T], dt.float16, tag="z", bufs=2)
                nc.vector.tensor_mul(zk[0:64, :], W2[0:64, 2 * c, :], W2[0:64, 2 * c, :])
                nc.vector.tensor_mul(zk[64:128, :], W2[64:128, 2 * c + 1, :],
                                     W2[64:128, 2 * c + 1, :])
                for n in range(2):
                    nc.tensor.matmul(b_ps[:, ts(n, 512)], E2k[:, c, :],
                                     zk[:, ts(n, 512)],
                                     start=False, stop=(c == 3))
            stg_b = pz.tile([8, T], dt.float16, tag="stg", bufs=1)
            nc.scalar.activation(out=stg_b, in_=b_ps, func=AF.Identity,
                                 scale=-1.0, bias=cb_sb[:, 1:2])
            nc.sync.dma_start(out=b_dr[:], in_=stg_b)
            nc.sync.dma_start(out=b1all, in_=b_dr[:, :])

            # ---- runs 5/6: vm, vc (normal layout, fp16, no bias) ----
            for i, (xi, wi, dest) in enumerate([(4, 2, vm16), (5, 3, vc16)]):
                xv = load_x(xi)
                wv = load_w(wi, i % 2)
                for m in range(NT):
                    ps = pps.tile([128, 512], dt.float32, tag="ps")
                    for k in range(4):
                        nc.tensor.matmul(ps, xv[k][:, ts(m, 128)],
                                         wv[:, k, :],
                                         start=(k == 0), stop=(k == 3))
                    if m % 2 == 0:
                        nc.scalar.activation(out=dest[:, m, :], in_=ps, func=AF.Copy)
                    else:
                        nc.vector.tensor_scalar(out=dest[:, m, :], in0=ps, scalar1=0.0,
                                                scalar2=None, op0=OP.add)

        # =================== P3 + pipelined P4 ===================
        g2 = [float(g) for g in gamma2]
        with tc.tile_pool(name="scp", bufs=2) as psc, \
             tc.tile_pool(name="ep", bufs=3) as pep, \
             tc.tile_pool(name="sxp", bufs=3) as psx, \
             tc.tile_pool(name="chp", bufs=4) as pch, \
             tc.tile_pool(name="dtp", bufs=2) as pdt, \
             tc.tile_pool(name="p2p", bufs=2) as pp2, \
             tc.tile_pool(name="p2t", bufs=3) as pt2, \
             tc.tile_pool(name="sqp", bufs=3) as psq, \
             tc.tile_pool(name="stgo", bufs=4) as pstg, \
             tc.tile_pool(name="p4c", bufs=2) as p4c, \
             tc.tile_pool(name="p4s", bufs=2) as p4s, \
             tc.tile_pool(name="tiny", bufs=24) as ptiny, \
             tc.tile_pool(name="ps_s", bufs=2, space="PSUM") as pps_s, \
             tc.tile_pool(name="ps_o", bufs=1, space="PSUM") as pps_o, \
             tc.tile_pool(name="ps_4", bufs=2, space="PSUM") as pps_4:

            for tb in range(NT):
                W = (tb + 1) * 128
                scp = psc.tile([128, H, T], dt.float16, tag="scp", name="scp")
                dts = pdt.tile([128, T], dt.float16, tag="dts", name="dts")
                nc.sync.dma_start(out=dts[:, :W], in_=dtab[:, DOFF[tb]:DOFF[tb] + W])
                for h in range(H):
                    nchunks = [(0, min(W, 512))] + ([(512, W)] if W > 512 else [])
                    ps = pps_s.tile([128, 1024], dt.float32, tag="ps_s")
                    for (s0, s1) in nchunks:
                        nc.tensor.matmul(ps[:, s0:s1], U2[:, h, ts(tb, 128)],
                                         W2[:, h, s0:s1], start=True, stop=False)
                        if s0 <= tb * 128 < s1:  # causal mask on the diag block
                            nc.tensor.matmul(ps[:, tb * 128:W], ident_sb, btri_sb,
                                             start=False, stop=False)
                        nc.tensor.matmul(ps[:, s0:s1], selh[:, h, :],
                                         b1all[:, s0:s1],
                                         start=False, stop=True)
                    # sc = 0.125*psum + alpha  (true scores, fp16);
                    # alternate ACT/DVE to balance engine load
                    if h < 4:
                        nc.scalar.activation(out=scp[:, h, :W], in_=ps[:, :W],
                                             func=AF.Identity, scale=0.125,
                                             bias=a1n8[:, tb, h:h + 1])
                    else:
                        nc.vector.tensor_scalar(out=scp[:, h, :W], in0=ps[:, :W],
                                                scalar1=0.125,
                                                scalar2=a1n8[:, tb, h:h + 1],
                                                op0=OP.mult, op1=OP.add)

                for half in range(2):
                    # e = exp(sc), bf16, batched over 4 heads
                    ep = pep.tile([128, 4, T], dt.bfloat16, tag="ep", name="ep")
                    nc.scalar.activation(out=ep[:, :, :W],
                                         in_=scp[:, 4 * half:4 * half + 4, :W],
                                         func=AF.Exp)
                    chn = pch.tile([128, 4, T], dt.float16, tag="chn", name="chn")
                    for hh in range(4):
                        h = 4 * half + hh
                        # reversed scan; incl-suffix[s'] lands at col s'+1 so the
                        # excl-suffix read starts at col 2
                        sx = psx.tile([128, 1026], dt.float16, tag="sx", name="sx")
                        nc.vector.memset(sx[:, W + 1:W + 2], 0.0)
                        esl = ep[:, hh, :W]
                        nc.vector.tensor_tensor_scan(out=rev(sx[:, 1:W + 1]),
                                                     data0=rev(esl), data1=rev(esl),
                                                     initial=0.0,
                                                     op0=OP.add, op1=OP.bypass)
                        rcp1 = ptiny.tile([128, 1], dt.float32, tag="rcp1")
                        nc.vector.reciprocal(out=rcp1, in_=sx[:, 1:2])
                        gr = ptiny.tile([128, 1], dt.float32, tag="gr")
                        nc.vector.tensor_scalar(out=gr, in0=rcp1, scalar1=g2[h],
                                                scalar2=None, op0=OP.mult)
                        # F'' = Sexcl[s] * (g^2/sm1) * |t-s|
                        nc.vector.scalar_tensor_tensor(out=chn[:, hh, :W],
                                                       in0=sx[:, 2:W + 2], scalar=gr,
                                                       in1=dts[:, :W],
                                                       op0=OP.mult, op1=OP.mult)
                    # ln-domain decay, batched over 4 heads, in-place
                    nc.scalar.activation(out=chn[:, :, :W], in_=chn[:, :, :W], func=AF.Ln)
                    nc.scalar.activation(out=chn[:, :, :W], in_=chn[:, :, :W], func=AF.Exp,
                                         scale=0.5)
                    nc.scalar.activation(out=chn[:, :, :W], in_=chn[:, :, :W], func=AF.Exp,
                                         scale=-1.0)
                    # tsc = te * sc (POOL), in-place into scp
                    for hh in range(4):
                        h = 4 * half + hh
                        nc.gpsimd.tensor_tensor(out=scp[:, h, :W], in0=chn[:, hh, :W],
                                                in1=scp[:, h, :W], op=OP.mult)
                    # second softmax + PV per head
                    for hh in range(4):
                        h = 4 * half + hh
                        half_p = (h % 2) * 64
                        pair = h // 2
                        if hh == 0:
                            om_g = pps_o.tile([128, 4, 128], dt.float32,
                                              tag=f"omg{half}", name=f"omg{half}")
                        om = om_g[:, 2 * (pair % 2):2 * (pair % 2) + 2, :]
                        p2u = pp2.tile([128, T], dt.float32, tag="p2u", name="p2u")
                        sm2 = ptiny.tile([128, 1], dt.float32, tag="sm2")
                        nc.scalar.activation(out=p2u[:, :W], in_=scp[:, h, :W],
                                             func=AF.Exp, accum_out=sm2)
                        rcp2 = ptiny.tile([128, 1], dt.float32, tag="rcp2")
                        nc.vector.reciprocal(out=rcp2, in_=sm2)
                        p2h = pp2.tile([128, T], dt.float16, tag="p2h", name="p2h")
                        nc.vector.tensor_scalar(out=p2h[:, :W], in0=p2u[:, :W],
                                                scalar1=rcp2, scalar2=None, op0=OP.mult)
                        p2T = pt2.tile([128, NT, 128], dt.float16, tag="p2T", name="p2T")
                        nc.sync.dma_start_transpose(out=p2T[:, 0:tb + 1, :], in_=p2h[:, :W])
                        p2sq = psq.tile([128, NT, 128], dt.float16, tag="p2sq", name="p2sq")
                        nc.gpsimd.tensor_mul(p2sq[:, 0:tb + 1, :], p2T[:, 0:tb + 1, :],
                                             p2T[:, 0:tb + 1, :])
                        hs = slice(h * DK, (h + 1) * DK)
                        # NOTE: the two accumulation groups share one PSUM bank and
                        # start=True clears has_written for the WHOLE bank, so the
                        # groups must be sequential, not interleaved.
                        for j in range(tb + 1):
                            nc.tensor.matmul(om[half_p:half_p + 64, 0, :],
                                             vm16[:, j, hs], p2T[:, j, :],
                                             start=(j == 0), stop=(j == tb))
                        for j in range(tb + 1):
                            nc.tensor.matmul(om[half_p:half_p + 64, 1, :],
                                             vc16[:, j, hs], p2sq[:, j, :],
                                             start=(j == 0), stop=(j == tb))
                        if h % 2 == 1:
                            st_m = pstg.tile([128, 128], dt.float16, tag="st_m")
                            nc.vector.tensor_scalar(out=st_m, in0=om[:, 0, :],
                                                    scalar1=bvp_sb[:, pair:pair + 1],
                                                    scalar2=None, op0=OP.add)
                            nc.sync.dma_start(
                                out=cm_drs[tb][0, 128 * pair:128 * (pair + 1), :],
                                in_=st_m)
                            st_c = pstg.tile([128, 128], dt.float16, tag="st_c")
                            nc.vector.tensor_scalar(out=st_c, in0=om[:, 1, :],
                                                    scalar1=bvp_sb[:, 4 + pair:5 + pair],
                                                    scalar2=None, op0=OP.add)
                            nc.sync.dma_start(
                                out=cm_drs[tb][1, 128 * pair:128 * (pair + 1), :],
                                in_=st_c)

                # ---- P4 for this row-block (pipelined with later tbs) ----
                for i, dst in enumerate([out_m, out_c]):
                    cmt = p4c.tile([128, 4, 128], dt.float16, tag="cmt")
                    nc.sync.dma_start(out=cmt,
                                      in_=cm_drs[tb][i].rearrange("(k p) t -> p k t",
                                                                  p=128))
                    ps4 = pps_4.tile([128, 512], dt.float32, tag="ps4")
                    for k in range(4):
                        nc.tensor.matmul(ps4, cmt[:, k, :], wo16[:, i, k, :],
                                         start=(k == 0), stop=False)
                    nc.tensor.matmul(ps4, ones16[0:1, ts(tb, 128)], bo16[0:1, i, :],
                                     start=False, stop=True)
                    st4 = p4s.tile([128, 512], dt.float32, tag="st4")
                    nc.vector.tensor_scalar(out=st4, in0=ps4, scalar1=0.0, scalar2=None,
                                            op0=OP.add)
                    nc.sync.dma_start(out=dst[ts(tb, 128), :], in_=st4)

    nc.finalize()
    return nc


def kernel(**inputs):
    f32 = lambda k: np.ascontiguousarray(np.asarray(inputs[k], np.float32))
    Wk, bk = f32('Wk_mean'), f32('bk_mean')
    Wkc, bkc = f32('Wk_cov'), f32('bk_cov')
    Wv, bv = f32('Wv_mean'), f32('bv_mean')
    Wvc, bvc = f32('Wv_cov'), f32('bv_cov')
    Wo, bo = f32('Wo_mean'), f32('bo_mean')
    Woc, boc = f32('Wo_cov'), f32('bo_cov')
    gammas = f32('gammas').reshape(H)
    zero_pad = int(np.asarray(inputs['zero_pad']))

    gamma = -np.log1p(np.exp(gammas))          # -softplus
    gamma2 = (gamma * gamma).astype(np.float64)

    # head-pair-swapped column permutation for the cov-side weights
    perm = np.arange(D).reshape(4, 2, DK)[:, ::-1, :].reshape(D)
    WkcT_sw = np.ascontiguousarray(Wkc.T[:, perm])
    bkc_sw = bkc[perm]

    wT = np.stack([np.ascontiguousarray(Wk.T), WkcT_sw,
                   np.ascontiguousarray(Wv.T), np.ascontiguousarray(Wvc.T)]).astype(F16)
    woT = np.stack([np.ascontiguousarray(Wo.T), np.ascontiguousarray(Woc.T)]).astype(F16)
    wc = np.ascontiguousarray(Wkc.T.reshape(D, H, DK).sum(-1)).astype(F16)  # [din, H]

    bqk = np.zeros((128, 12), np.float32)
    bqk[:, 0:4] = bk.reshape(4, 128).T
    bqk[:, 4:8] = bkc_sw.reshape(4, 128).T
    bqk[:, 8:12] = 2.0 * bk.reshape(4, 128).T
    bvo16 = np.stack([bo, boc]).astype(F16)
    bvp = np.concatenate([bv.reshape(4, 128).T, bvc.reshape(4, 128).T], axis=1).astype(np.float32)
    sb = bkc.reshape(H, DK).sum(-1)
    cbt = np.stack([sb, -sb], axis=1).astype(np.float32)         # [8, 2]
    cbr = np.ascontiguousarray((-0.125 * sb)[None, :]).astype(np.float32)  # [1, 8]

    btri = np.triu(np.full((128, 128), NEG, np.float32), 1).astype(F16)
    ident = np.eye(128, dtype=F16)
    idx_t = np.arange(T)
    dtab = np.zeros((128, DD_COLS), np.float32)
    for tb in range(NT):
        tt = tb * 128 + np.arange(128)
        W = (tb + 1) * 128
        dtab[:, DOFF[tb]:DOFF[tb] + W] = np.abs(tt[:, None] - idx_t[None, :W])
    dtab = dtab.astype(F16)

    selh_np = np.zeros((8, H, 128), np.float32)
    for h in range(H):
        selh_np[h, h, :] = 1.0
    selh_np = selh_np.reshape(8, H * 128).astype(F16)

    xs = [f32('q_mean'), f32('q_cov'), f32('k_mean'), f32('k_cov'),
          f32('v_mean'), f32('v_cov')]

    nc = _build(gamma2)

    in_maps = []
    for b in range(B):
        xTb = np.stack([np.ascontiguousarray(x[b].T) for x in xs]).astype(F16)
        in_maps.append(dict(xT=xTb, wT=wT, woT=woT, wc=wc, bqk=bqk, bvp=bvp,
                            bvo16=bvo16, cb=cbt, cbr=cbr, btri=btri, ident=ident,
                            dtab=dtab, selh_in=selh_np))

    from concourse.bass_utils import run_bass_kernel_spmd
    trace = bool(int(os.environ.get("KERNEL_TRACE", "0")))
    res = run_bass_kernel_spmd(nc, in_maps, list(range(B)), trace=trace)
    if trace and res.exec_time_ns is not None:
        print(f"HW exec time: {res.exec_time_ns} ns")
        if res.mean_exec_time_ns is not None:
            print(f"HW exec time mean: {res.mean_exec_time_ns:.0f} ns")

    out_mean = np.stack([res.results[b]["out_m"] for b in range(B)])
    out_cov = np.stack([res.results[b]["out_c"] for b in range(B)])
    if zero_pad:
        out_mean[:, 0, :] = bo[None, :]
        out_cov[:, 0, :] = boc[None, :]
    return out_mean, out_cov


# revision 26
# speedup vs baseline: 1.0235x; 1.0235x over previous
"""Trainium2 Bass kernel for nn_MultiHeadAttention_41944650612760.

Wasserstein-distance multi-head attention with cumulative position decay.
Sharding: data-parallel over batch B=8 across 8 NeuronCores (one batch/core).

Design notes:
  - fp16 matmuls everywhere (P1 projections, scores, PV, P4).
  - one ACT table set per phase: P1 issues all Sqrt work first, then only
    filler funcs (Identity/Square/Copy), so the switch to P3's
    natural_log_exp_and_others set happens exactly once; the position-decay
    sqrt runs in the ln-domain (sqrt(x) = exp(0.5 ln x)).
  - causal btri mask added on PE via identity-matmul; b1 column term added
    via K=1 ones-matmul; both folded into the scores PSUM accumulation.
  - suffix mass S computed directly by a REVERSED DVE scan (no sm1 - C
    cancellation), fp16 out; F'' = S * (g^2/sm1) * |t-s| in one fused stt.
  - decay chain (Ln, Exp, Exp) batched across 4-head halves.
  - second-softmax numerator kept fp32 until normalize, then fp16 for the
    transpose + PV; PV accumulates per row-block into packed PSUM slots
    (mean and cov groups sequential: start=True clears the whole bank).
  - P4 output projection runs per row-block, pipelined with P3 via per-tb
    DRAM bounce tiles.

zero_pad is applied on the host (row 0 of each output = bias).
"""

import os
import numpy as np
from contextlib import ExitStack

B, T, D, H = 8, 1024, 512, 8
DK = D // H          # 64
NT = T // 128        # 8 row/col blocks
NEG = -30000.0       # fp16-safe mask constant (x0.125 => -3750)
F16 = np.float16

# packed row-block layout for the |t-s| table: block tb holds s in
# [0, (tb+1)*128) at DOFF[tb]
DOFF = [0] * NT
for _t in range(1, NT):
    DOFF[_t] = DOFF[_t - 1] + _t * 128
DD_COLS = DOFF[-1] + NT * 128  # 4608


def _build(gamma2):
    """Trace the Bass program. gamma2[h] = gamma_h**2 (trace-time floats)."""
    import concourse.bass as bass
    import concourse.bacc as bacc
    import concourse.mybir as mybir
    import concourse.tile as tile

    # Steer the ACT table chooser: drop `exp` from exp_and_others and `ln`
    # from natural_log so both resolve to natural_log_exp_and_others (one
    # resident set for all of P3/P4 instead of per-Ln flip-flops). Set IDs
    # stay positional, so the emitted act_func_set_id values remain valid.
    import concourse.hw_specs as hw_specs
    if not getattr(bacc, "_act_tbl_patched", False):
        _orig_gat = hw_specs.get_activation_tables

        def _gat(arch):
            tables = _orig_gat(arch)
            exp_t = mybir.ActivationFunctionType.from_pwp("exp")
            ln_t = mybir.ActivationFunctionType.from_pwp("ln")
            if "natural_log_exp_and_others" in tables:
                tables.get("exp_and_others", set()).discard(exp_t)
                tables.get("natural_log", set()).discard(ln_t)
            return tables

        bacc.get_activation_tables = _gat
        bacc._act_tbl_patched = True

    dt = mybir.dt
    AF = mybir.ActivationFunctionType
    OP = mybir.AluOpType
    ts = bass.ts

    nc = bacc.Bacc()

    # ---- per-core DRAM I/O ----
    xT = nc.declare_dram_parameter("xT", [6, D, T], dt.float16, isOutput=False)
    wT = nc.declare_dram_parameter("wT", [4, D, D], dt.float16, isOutput=False)
    woT = nc.declare_dram_parameter("woT", [2, D, D], dt.float16, isOutput=False)
    wc = nc.declare_dram_parameter("wc", [D, H], dt.float16, isOutput=False)
    bqk = nc.declare_dram_parameter("bqk", [128, 12], dt.float32, isOutput=False)
    bvp = nc.declare_dram_parameter("bvp", [128, 8], dt.float32, isOutput=False)
    bvo16 = nc.declare_dram_parameter("bvo16", [2, D], dt.float16, isOutput=False)
    cb = nc.declare_dram_parameter("cb", [8, 2], dt.float32, isOutput=False)
    cbr = nc.declare_dram_parameter("cbr", [1, 8], dt.float32, isOutput=False)
    btri = nc.declare_dram_parameter("btri", [128, 128], dt.float16, isOutput=False)
    ident = nc.declare_dram_parameter("ident", [128, 128], dt.float16, isOutput=False)
    dtab = nc.declare_dram_parameter("dtab", [128, DD_COLS], dt.float16, isOutput=False)
    selh_in = nc.declare_dram_parameter("selh_in", [8, H * 128], dt.float16, isOutput=False)
    out_m = nc.declare_dram_parameter("out_m", [T, D], dt.float32, isOutput=True)
    out_c = nc.declare_dram_parameter("out_c", [T, D], dt.float32, isOutput=True)

    def rev(a):
        """Reverse an AP [128, W] along its (last) free dim."""
        st, cnt = a.ap[-1]
        return bass.AP(tensor=a.tensor, offset=a.offset + (cnt - 1) * st,
                       ap=list(a.ap[:-1]) + [[-st, cnt]])

    with tile.TileContext(nc) as tc, ExitStack() as ctx:
        pc = ctx.enter_context(tc.tile_pool(name="pc", bufs=1))
        pdr = ctx.enter_context(tc.tile_pool(name="pdr", bufs=1, space="DRAM"))

        # ---- persistent SBUF tensors ----
        U2 = pc.tile([128, H, T], dt.float16)       # [qm_h ; sqq_h] per head (parity split)
        W2 = pc.tile([128, H, T], dt.float16)       # [2km_h ; 2sqk_h]
        vm16 = pc.tile([128, NT, D], dt.float16)    # vm normal layout fp16
        vc16 = pc.tile([128, NT, D], dt.float16)
        btri_sb = pc.tile([128, 128], dt.float16)
        ident_sb = pc.tile([128, 128], dt.float16)
        ones16 = pc.tile([1, T], dt.float16)
        a1n8 = pc.tile([128, NT, 8], dt.float32)    # -0.125 * (a1[t]+sb) per (tb, h)
        cbn8 = pc.tile([128, 8], dt.float32)        # -0.125 * sb, bcast to 128 parts
        bqk_sb = pc.tile([128, 12], dt.float32)     # [bk|bkc_sw|2bk] x 4 chunks
        bvp_sb = pc.tile([128, 8], dt.float32)      # bv, bvc pair-sliced
        cb_sb = pc.tile([8, 2], dt.float32)
        wc_sb = pc.tile([128, 4, H], dt.float16)
        E2q = pc.tile([128, 4, 8], dt.float16)
        E2k = pc.tile([128, 4, 8], dt.float16)
        wo16 = pc.tile([128, 2, 4, D], dt.float16)  # P4 weights, both outputs
        bo16 = pc.tile([1, 2, D], dt.float16)
        b1all = pc.tile([8, T], dt.float16)         # -b1 rows, partitions 0-7
        selh = pc.tile([8, H, 128], dt.float16)     # one-hot row selectors (K=8 MM)

        b_dr = pdr.tile([H, T], dt.float16)         # -b1 rows via DRAM bounce
        cm_drs = [pdr.tile([2, D, 128], dt.float16, name=f"cm{tb}")
                  for tb in range(NT)]              # per-tb attention-out bounce

        nc.sync.dma_start(out=btri_sb, in_=btri[:, :])
        nc.sync.dma_start(out=ident_sb, in_=ident[:, :])
        nc.sync.dma_start(out=bqk_sb, in_=bqk[:, :])
        nc.sync.dma_start(out=bvp_sb, in_=bvp[:, :])
        nc.sync.dma_start(out=cb_sb, in_=cb[:, :])
        nc.sync.dma_start(out=wc_sb, in_=wc.rearrange("(k p) h -> p k h", p=128))
        nc.sync.dma_start(out=wo16, in_=woT.rearrange("i (k p) d -> p i k d", p=128))
        _bv = bvo16[:, :]
        nc.sync.dma_start(out=bo16, in_=bass.AP(tensor=_bv.tensor, offset=_bv.offset,
                                                ap=[[0, 1]] + list(_bv.ap)))
        nc.vector.memset(ones16, 1.0)
        _cbr = cbr[0:1, :]
        nc.sync.dma_start(out=cbn8, in_=bass.AP(tensor=_cbr.tensor, offset=_cbr.offset,
                                                ap=[[0, 128]] + list(_cbr.ap[1:])))
        nc.sync.dma_start(out=selh, in_=selh_in.rearrange("p (h c) -> p h c", h=H))
        nc.vector.memset(E2q, 0.0)
        nc.vector.memset(E2k, 0.0)
        for c in range(4):
            nc.vector.memset(E2q[0:64, c, 2 * c:2 * c + 1], 1.0)
            nc.vector.memset(E2q[64:128, c, 2 * c + 1:2 * c + 2], 1.0)
            nc.vector.memset(E2k[0:64, c, 2 * c:2 * c + 1], 0.25)
            nc.vector.memset(E2k[64:128, c, 2 * c + 1:2 * c + 2], 0.25)

        # =================== P1: projections + P2: a1/b1 ===================
        # Order: cov runs (2, 4) FIRST so all ACT Sqrt work precedes the
        # P3 exp/ln table set (exactly one ACT_TABLE_LOAD switch).
        with tc.tile_pool(name="p1x", bufs=6) as px, \
             tc.tile_pool(name="p1w", bufs=1) as pw, \
             tc.tile_pool(name="p1z", bufs=3) as pz, \
             tc.tile_pool(name="p1ps", bufs=2, space="PSUM") as pps, \
             tc.tile_pool(name="p1pa", bufs=1, space="PSUM") as ppa:

            b_ps = ppa.tile([8, T], dt.float32, tag="b_ps")   # b1 = m2sq + kcs
            a1t_ps = ppa.tile([128, NT, 8], dt.float32, tag="a1t")  # a1 in [t, (tb,h)]

            def load_x(i):
                xs = []
                for k in range(4):
                    xt = px.tile([128, T], dt.float16, tag="xt")
                    nc.sync.dma_start(out=xt, in_=xT[i, ts(k, 128), :])
                    xs.append(xt)
                return xs

            def load_w(i, tagslot):
                wt = pw.tile([128, 4, D], dt.float16, tag=f"wt{tagslot}")
                nc.sync.dma_start(out=wt, in_=wT[i].rearrange("(k p) d -> p k d", p=128))
                return wt

            # ---- run 2: sqq -> U2 (clip+sqrt) + qcs into a1t (group start) ----
            xqc = load_x(1)
            wkc = load_w(1, 1)
            for c in range(4):
                for n in range(2):
                    ps = pps.tile([128, 512], dt.float32, tag="ps")
                    for k in range(4):
                        nc.tensor.matmul(ps, wkc[:, k, ts(c, 128)],
                                         xqc[k][:, ts(n, 512)],
                                         start=(k == 0), stop=(k == 3))
                    nc.vector.tensor_scalar(out=ps, in0=ps, scalar1=bqk_sb[:, 4 + c:5 + c],
                                            scalar2=1e-24, op0=OP.add, op1=OP.max)
                    # swapped col order: psum[0:64] = head 2c+1 (odd -> low parts)
                    nc.scalar.activation(out=U2[0:64, 2 * c + 1, ts(n, 512)], in_=ps[0:64],
                                         func=AF.Sqrt)
                    nc.scalar.activation(out=U2[64:128, 2 * c, ts(n, 512)], in_=ps[64:128],
                                         func=AF.Sqrt)
            # single accumulation group for the whole a1t bank: start=True
            # clears has_written bank-wide, so only the very first MM starts
            for tb in range(NT):
                for k in range(4):
                    nc.tensor.matmul(a1t_ps[:, tb, :], xqc[k][:, ts(tb, 128)],
                                     wc_sb[:, k, :],
                                     start=(tb == 0 and k == 0), stop=False)

            # ---- run 4: 2sqk -> W2 + kcs into b_ps (group start) ----
            xkc = load_x(3)
            for n in range(2):
                for k in range(4):
                    nc.tensor.matmul(b_ps[:, ts(n, 512)], wc_sb[:, k, :],
                                     xkc[k][:, ts(n, 512)],
                                     start=(k == 0), stop=False)
            for c in range(4):
                for n in range(2):
                    ps = pps.tile([128, 512], dt.float32, tag="ps")
                    for k in range(4):
                        nc.tensor.matmul(ps, wkc[:, k, ts(c, 128)],
                                         xkc[k][:, ts(n, 512)],
                                         start=(k == 0), stop=(k == 3))
                    nc.vector.tensor_scalar(out=ps, in0=ps, scalar1=bqk_sb[:, 4 + c:5 + c],
                                            scalar2=1e-24, op0=OP.add, op1=OP.max)
                    nc.scalar.activation(out=W2[0:64, 2 * c + 1, ts(n, 512)], in_=ps[0:64],
                                         func=AF.Sqrt, scale=4.0)
                    nc.scalar.activation(out=W2[64:128, 2 * c, ts(n, 512)], in_=ps[64:128],
                                         func=AF.Sqrt, scale=4.0)

            # ---- run 1: qm -> U2 (bias bk; ACT/DVE split) ----
            xq = load_x(0)
            wk = load_w(0, 0)
            for c in range(4):
                for n in range(2):
                    ps = pps.tile([128, 512], dt.float32, tag="ps")
                    for k in range(4):
                        nc.tensor.matmul(ps, wk[:, k, ts(c, 128)],
                                         xq[k][:, ts(n, 512)],
                                         start=(k == 0), stop=(k == 3))
                    if n == 0:
                        nc.scalar.activation(out=U2[0:64, 2 * c, ts(n, 512)], in_=ps[0:64],
                                             func=AF.Identity, bias=bqk_sb[0:64, c:c + 1])
                        nc.scalar.activation(out=U2[64:128, 2 * c + 1, ts(n, 512)],
                                             in_=ps[64:128],
                                             func=AF.Identity, bias=bqk_sb[64:128, c:c + 1])
                    else:
                        nc.vector.tensor_scalar(out=U2[0:64, 2 * c, ts(n, 512)],
                                                in0=ps[0:64], scalar1=bqk_sb[0:64, c:c + 1],
                                                scalar2=None, op0=OP.add)
                        nc.vector.tensor_scalar(out=U2[64:128, 2 * c + 1, ts(n, 512)],
                                                in0=ps[64:128],
                                                scalar1=bqk_sb[64:128, c:c + 1],
                                                scalar2=None, op0=OP.add)

            # ---- run 3: 2km -> W2 (bias 2bk, scale 2; ACT/DVE split) ----
            xk = load_x(2)
            for c in range(4):
                for n in range(2):
                    ps = pps.tile([128, 512], dt.float32, tag="ps")
                    for k in range(4):
                        nc.tensor.matmul(ps, wk[:, k, ts(c, 128)],
                                         xk[k][:, ts(n, 512)],
                                         start=(k == 0), stop=(k == 3))
                    if n == 0:
                        nc.scalar.activation(out=W2[0:64, 2 * c, ts(n, 512)], in_=ps[0:64],
                                             func=AF.Identity, scale=2.0,
                                             bias=bqk_sb[0:64, 8 + c:9 + c])
                        nc.scalar.activation(out=W2[64:128, 2 * c + 1, ts(n, 512)],
                                             in_=ps[64:128], func=AF.Identity, scale=2.0,
                                             bias=bqk_sb[64:128, 8 + c:9 + c])
                    else:
                        nc.vector.tensor_scalar(out=W2[0:64, 2 * c, ts(n, 512)],
                                                in0=ps[0:64], scalar1=2.0,
                                                scalar2=bqk_sb[0:64, 8 + c:9 + c],
                                                op0=OP.mult, op1=OP.add)
                        nc.vector.tensor_scalar(out=W2[64:128, 2 * c + 1, ts(n, 512)],
                                                in0=ps[64:128], scalar1=2.0,
                                                scalar2=bqk_sb[64:128, 8 + c:9 + c],
                                                op0=OP.mult, op1=OP.add)

            # ---- q-side squares (m1sq) -> finish a1t group; evict a1n8 ----
            zqs = []
            for c in range(4):
                zq = pz.tile([128, T], dt.float16, tag=f"zq{c}", name="zq", bufs=1)
                nc.scalar.activation(out=zq[0:64, :], in_=U2[0:64, 2 * c, :], func=AF.Square)
                nc.scalar.activation(out=zq[64:128, :], in_=U2[64:128, 2 * c + 1, :],
                                     func=AF.Square)
                zqs.append(zq)
            for tb in range(NT):
                for c in range(4):
                    nc.tensor.matmul(a1t_ps[:, tb, :], zqs[c][:, ts(tb, 128)],
                                     E2q[:, c, :],
                                     start=False, stop=(tb == NT - 1 and c == 3))
            nc.scalar.activation(out=a1n8, in_=a1t_ps, func=AF.Copy, scale=-0.125)
            nc.vector.tensor_tensor(out=a1n8, in0=a1n8,
                                    in1=bass.AP(tensor=cbn8.tensor, offset=cbn8.offset,
                                                ap=[cbn8.ap[0], [0, NT]] + list(cbn8.ap[1:])),
                                    op=OP.add)

            # ---- k-side squares (of 2km; E2k carries the 1/4) -> b_ps; b1 rows ----
            for c in range(4):
                zk = pz.tile([128, T], dt.float16, tag="z", bufs=2)
                nc.vector.tensor_mul(zk[0:64, :], W2[0:64, 2 * c, :], W2[0:64, 2 * c, :])
                nc.vector.tensor_mul(zk[64:128, :], W2[64:128, 2 * c + 1, :],
                                     W2[64:128, 2 * c + 1, :])
                for n in range(2):
                    nc.tensor.matmul(b_ps[:, ts(n, 512)], E2k[:, c, :],
                                     zk[:, ts(n, 512)],
                                     start=False, stop=(c == 3))
            stg_b = pz.tile([8, T], dt.float16, tag="stg", bufs=1)
            nc.scalar.activation(out=stg_b, in_=b_ps, func=AF.Identity,
                                 scale=-1.0, bias=cb_sb[:, 1:2])
            nc.sync.dma_start(out=b_dr[:], in_=stg_b)
            nc.sync.dma_start(out=b1all, in_=b_dr[:, :])

            # ---- runs 5/6: vm, vc (normal layout, fp16, no bias) ----
            for i, (xi, wi, dest) in enumerate([(4, 2, vm16), (5, 3, vc16)]):
                xv = load_x(xi)
                wv = load_w(wi, i % 2)
                for m in range(NT):
                    ps = pps.tile([128, 512], dt.float32, tag="ps")
                    for k in range(4):
                        nc.tensor.matmul(ps, xv[k][:, ts(m, 128)],
                                         wv[:, k, :],
                                         start=(k == 0), stop=(k == 3))
                    if m % 2 == 0:
                        nc.scalar.activation(out=dest[:, m, :], in_=ps, func=AF.Copy)
                    else:
                        nc.vector.tensor_scalar(out=dest[:, m, :], in0=ps, scalar1=0.0,
                                                scalar2=None, op0=OP.add)

        # =================== P3 + pipelined P4 ===================
        g2 = [float(g) for g in gamma2]
        with tc.tile_pool(name="scp", bufs=4) as psc, \
             tc.tile_pool(name="ep", bufs=3) as pep, \
             tc.tile_pool(name="sxp", bufs=4) as psx, \
             tc.tile_pool(name="chp", bufs=4) as pch, \
             tc.tile_pool(name="dtp", bufs=2) as pdt, \
             tc.tile_pool(name="p2p", bufs=2) as pp2, \
             tc.tile_pool(name="p2t", bufs=3) as pt2, \
             tc.tile_pool(name="sqp", bufs=3) as psq, \
             tc.tile_pool(name="stgo", bufs=4) as pstg, \
             tc.tile_pool(name="p4c", bufs=2) as p4c, \
             tc.tile_pool(name="p4s", bufs=2) as p4s, \
             tc.tile_pool(name="tiny", bufs=24) as ptiny, \
             tc.tile_pool(name="ps_s", bufs=4, space="PSUM") as pps_s, \
             tc.tile_pool(name="ps_o", bufs=1, space="PSUM") as pps_o, \
             tc.tile_pool(name="ps_4", bufs=2, space="PSUM") as pps_4:

            for tb in range(NT):
                W = (tb + 1) * 128
                dts = pdt.tile([128, T], dt.float16, tag="dts", name="dts")
                nc.sync.dma_start(out=dts[:, :W], in_=dtab[:, DOFF[tb]:DOFF[tb] + W])
                for half in range(2):
                    scp = psc.tile([128, 4, T], dt.float16, tag="scp", name="scp")
                    for hh in range(4):
                        h = 4 * half + hh
                        nchunks = [(0, min(W, 512))] + ([(512, W)] if W > 512 else [])
                        for (s0, s1) in nchunks:
                            ps = pps_s.tile([128, 512], dt.float32, tag="ps_s")
                            pw = s1 - s0
                            nc.tensor.matmul(ps[:, 0:pw], U2[:, h, ts(tb, 128)],
                                             W2[:, h, s0:s1], start=True, stop=False)
                            if s0 <= tb * 128 < s1:  # causal mask on the diag block
                                nc.tensor.matmul(ps[:, tb * 128 - s0:pw], ident_sb,
                                                 btri_sb, start=False, stop=False)
                            nc.tensor.matmul(ps[:, 0:pw], selh[:, h, :],
                                             b1all[:, s0:s1],
                                             start=False, stop=True)
                            # sc = 0.125*psum + alpha  (true scores, fp16);
                            # alternate ACT/DVE to balance engine load
                            if h < 4:
                                nc.scalar.activation(out=scp[:, hh, s0:s1],
                                                     in_=ps[:, 0:pw],
                                                     func=AF.Identity, scale=0.125,
                                                     bias=a1n8[:, tb, h:h + 1])
                            else:
                                nc.vector.tensor_scalar(out=scp[:, hh, s0:s1],
                                                        in0=ps[:, 0:pw],
                                                        scalar1=0.125,
                                                        scalar2=a1n8[:, tb, h:h + 1],
                                                        op0=OP.mult, op1=OP.add)

                    # e = exp(sc), bf16, batched over 4 heads
                    ep = pep.tile([128, 4, T], dt.bfloat16, tag="ep", name="ep")
                    nc.scalar.activation(out=ep[:, :, :W],
                                         in_=scp[:, :, :W],
                                         func=AF.Exp)
                    chn = pch.tile([128, 4, T], dt.float16, tag="chn", name="chn")
                    for hh in range(4):
                        h = 4 * half + hh
                        # reversed scan; incl-suffix[s'] lands at col s'+1 so the
                        # excl-suffix read starts at col 2
                        sx = psx.tile([128, 1026], dt.float16, tag="sx", name="sx")
                        nc.vector.memset(sx[:, W + 1:W + 2], 0.0)
                        esl = ep[:, hh, :W]
                        nc.vector.tensor_tensor_scan(out=rev(sx[:, 1:W + 1]),
                                                     data0=rev(esl), data1=rev(esl),
                                                     initial=0.0,
                                                     op0=OP.add, op1=OP.bypass)
                        rcp1 = ptiny.tile([128, 1], dt.float32, tag="rcp1")
                        nc.vector.reciprocal(out=rcp1, in_=sx[:, 1:2])
                        gr = ptiny.tile([128, 1], dt.float32, tag="gr")
                        nc.vector.tensor_scalar(out=gr, in0=rcp1, scalar1=g2[h],
                                                scalar2=None, op0=OP.mult)
                        # F'' = Sexcl[s] * (g^2/sm1) * |t-s|
                        nc.vector.scalar_tensor_tensor(out=chn[:, hh, :W],
                                                       in0=sx[:, 2:W + 2], scalar=gr,
                                                       in1=dts[:, :W],
                                                       op0=OP.mult, op1=OP.mult)
                    # ln-domain decay, batched over 4 heads, in-place
                    nc.scalar.activation(out=chn[:, :, :W], in_=chn[:, :, :W], func=AF.Ln)
                    nc.scalar.activation(out=chn[:, :, :W], in_=chn[:, :, :W], func=AF.Exp,
                                         scale=0.5)
                    nc.scalar.activation(out=chn[:, :, :W], in_=chn[:, :, :W], func=AF.Exp,
                                         scale=-1.0)
                    # tsc = te * sc (POOL), in-place into scp
                    for hh in range(4):
                        nc.gpsimd.tensor_tensor(out=scp[:, hh, :W], in0=chn[:, hh, :W],
                                                in1=scp[:, hh, :W], op=OP.mult)
                    # second softmax + PV per head
                    for hh in range(4):
                        h = 4 * half + hh
                        half_p = (h % 2) * 64
                        pair = h // 2
                        if hh == 0:
                            om_g = pps_o.tile([128, 4, 128], dt.float32,
                                              tag=f"omg{half}", name=f"omg{half}")
                        om = om_g[:, 2 * (pair % 2):2 * (pair % 2) + 2, :]
                        p2u = pp2.tile([128, T], dt.float32, tag="p2u", name="p2u")
                        sm2 = ptiny.tile([128, 1], dt.float32, tag="sm2")
                        nc.scalar.activation(out=p2u[:, :W], in_=scp[:, hh, :W],
                                             func=AF.Exp, accum_out=sm2)
                        rcp2 = ptiny.tile([128, 1], dt.float32, tag="rcp2")
                        nc.vector.reciprocal(out=rcp2, in_=sm2)
                        p2h = pp2.tile([128, T], dt.float16, tag="p2h", name="p2h")
                        nc.vector.tensor_scalar(out=p2h[:, :W], in0=p2u[:, :W],
                                                scalar1=rcp2, scalar2=None, op0=OP.mult)
                        p2T = pt2.tile([128, NT, 128], dt.float16, tag="p2T", name="p2T")
                        nc.sync.dma_start_transpose(out=p2T[:, 0:tb + 1, :], in_=p2h[:, :W])
                        p2sq = psq.tile([128, NT, 128], dt.float16, tag="p2sq", name="p2sq")
                        nc.gpsimd.tensor_mul(p2sq[:, 0:tb + 1, :], p2T[:, 0:tb + 1, :],
                                             p2T[:, 0:tb + 1, :])
                        hs = slice(h * DK, (h + 1) * DK)
                        # NOTE: the two accumulation groups share one PSUM bank and
                        # start=True clears has_written for the WHOLE bank, so the
                        # groups must be sequential, not interleaved.
                        for j in range(tb + 1):
                            nc.tensor.matmul(om[half_p:half_p + 64, 0, :],
                                             vm16[:, j, hs], p2T[:, j, :],
                                             start=(j == 0), stop=(j == tb))
                        for j in range(tb + 1):
                            nc.tensor.matmul(om[half_p:half_p + 64, 1, :],
                                             vc16[:, j, hs], p2sq[:, j, :],
                                             start=(j == 0), stop=(j == tb))
                        if h % 2 == 1:
                            st_m = pstg.tile([128, 128], dt.float16, tag="st_m")
                            nc.vector.tensor_scalar(out=st_m, in0=om[:, 0, :],
                                                    scalar1=bvp_sb[:, pair:pair + 1],
                                                    scalar2=None, op0=OP.add)
                            nc.sync.dma_start(
                                out=cm_drs[tb][0, 128 * pair:128 * (pair + 1), :],
                                in_=st_m)
                            st_c = pstg.tile([128, 128], dt.float16, tag="st_c")
                            nc.vector.tensor_scalar(out=st_c, in0=om[:, 1, :],
                                                    scalar1=bvp_sb[:, 4 + pair:5 + pair],
                                                    scalar2=None, op0=OP.add)
                            nc.sync.dma_start(
                                out=cm_drs[tb][1, 128 * pair:128 * (pair + 1), :],
                                in_=st_c)

                # ---- P4 for this row-block (pipelined with later tbs) ----
                for i, dst in enumerate([out_m, out_c]):
                    cmt = p4c.tile([128, 4, 128], dt.float16, tag="cmt")
                    nc.sync.dma_start(out=cmt,
                                      in_=cm_drs[tb][i].rearrange("(k p) t -> p k t",
                                                                  p=128))
                    ps4 = pps_4.tile([128, 512], dt.float32, tag="ps4")
                    for k in range(4):
                        nc.tensor.matmul(ps4, cmt[:, k, :], wo16[:, i, k, :],
                                         start=(k == 0), stop=False)
                    nc.tensor.matmul(ps4, ones16[0:1, ts(tb, 128)], bo16[0:1, i, :],
                                     start=False, stop=True)
                    st4 = p4s.tile([128, 512], dt.float32, tag="st4")
                    nc.vector.tensor_scalar(out=st4, in0=ps4, scalar1=0.0, scalar2=None,
                                            op0=OP.add)
                    nc.sync.dma_start(out=dst[ts(tb, 128), :], in_=st4)

    nc.finalize()
    return nc


def kernel(**inputs):
    f32 = lambda k: np.ascontiguousarray(np.asarray(inputs[k], np.float32))
    Wk, bk = f32('Wk_mean'), f32('bk_mean')
    Wkc, bkc = f32('Wk_cov'), f32('bk_cov')
    Wv, bv = f32('Wv_mean'), f32('bv_mean')
    Wvc, bvc = f32('Wv_cov'), f32('bv_cov')
    Wo, bo = f32('Wo_mean'), f32('bo_mean')
    Woc, boc = f32('Wo_cov'), f32('bo_cov')
    gammas = f32('gammas').reshape(H)
    zero_pad = int(np.asarray(inputs['zero_pad']))

    gamma = -np.log1p(np.exp(gammas))          # -softplus
    gamma2 = (gamma * gamma).astype(np.float64)

    # head-pair-swapped column permutation for the cov-side weights
    perm = np.arange(D).reshape(4, 2, DK)[:, ::-1, :].reshape(D)
    WkcT_sw = np.ascontiguousarray(Wkc.T[:, perm])
    bkc_sw = bkc[perm]

    wT = np.stack([np.ascontiguousarray(Wk.T), WkcT_sw,
                   np.ascontiguousarray(Wv.T), np.ascontiguousarray(Wvc.T)]).astype(F16)
    woT = np.stack([np.ascontiguousarray(Wo.T), np.ascontiguousarray(Woc.T)]).astype(F16)
    wc = np.ascontiguousarray(Wkc.T.reshape(D, H, DK).sum(-1)).astype(F16)  # [din, H]

    bqk = np.zeros((128, 12), np.float32)
    bqk[:, 0:4] = bk.reshape(4, 128).T
    bqk[:, 4:8] = bkc_sw.reshape(4, 128).T
    bqk[:, 8:12] = 2.0 * bk.reshape(4, 128).T
    bvo16 = np.stack([bo, boc]).astype(F16)
    bvp = np.concatenate([bv.reshape(4, 128).T, bvc.reshape(4, 128).T], axis=1).astype(np.float32)
    sb = bkc.reshape(H, DK).sum(-1)
    cbt = np.stack([sb, -sb], axis=1).astype(np.float32)         # [8, 2]
    cbr = np.ascontiguousarray((-0.125 * sb)[None, :]).astype(np.float32)  # [1, 8]

    btri = np.triu(np.full((128, 128), NEG, np.float32), 1).astype(F16)
    ident = np.eye(128, dtype=F16)
    idx_t = np.arange(T)
    dtab = np.zeros((128, DD_COLS), np.float32)
    for tb in range(NT):
        tt = tb * 128 + np.arange(128)
        W = (tb + 1) * 128
        dtab[:, DOFF[tb]:DOFF[tb] + W] = np.abs(tt[:, None] - idx_t[None, :W])
    dtab = dtab.astype(F16)

    selh_np = np.zeros((8, H, 128), np.float32)
    for h in range(H):
        selh_np[h, h, :] = 1.0
    selh_np = selh_np.reshape(8, H * 128).astype(F16)

    xs = [f32('q_mean'), f32('q_cov'), f32('k_mean'), f32('k_cov'),
          f32('v_mean'), f32('v_cov')]

    nc = _build(gamma2)

    in_maps = []
    for b in range(B):
        xTb = np.stack([np.ascontiguousarray(x[b].T) for x in xs]).astype(F16)
        in_maps.append(dict(xT=xTb, wT=wT, woT=woT, wc=wc, bqk=bqk, bvp=bvp,
                            bvo16=bvo16, cb=cbt, cbr=cbr, btri=btri, ident=ident,
                            dtab=dtab, selh_in=selh_np))

    from concourse.bass_utils import run_bass_kernel_spmd
    trace = bool(int(os.environ.get("KERNEL_TRACE", "0")))
    res = run_bass_kernel_spmd(nc, in_maps, list(range(B)), trace=trace)
    if trace and res.exec_time_ns is not None:
        print(f"HW exec time: {res.exec_time_ns} ns")
        if res.mean_exec_time_ns is not None:
            print(f"HW exec time mean: {res.mean_exec_time_ns:.0f} ns")

    out_mean = np.stack([res.results[b]["out_m"] for b in range(B)])
    out_cov = np.stack([res.results[b]["out_c"] for b in range(B)])
    if zero_pad:
        out_mean[:, 0, :] = bo[None, :]
        out_cov[:, 0, :] = boc[None, :]
    return out_mean, out_cov
